# revision 8
# baseline (speedup 1.0000x reference)
"""CardEncoder Trainium2 kernel — v6.

v6: the embedding gather moves to the host.  kernel() pre-gathers, time-
reverses, and lays out the embeddings tile-by-tile in the exact SBUF
layout the LSTM needs; the device streams contiguous 2MB tiles from HBM
(HWDGE, full bandwidth) instead of issuing 262k scatter-gather
descriptors.  Everything below the embed step is v5:

Changes vs v4:
  * Two HBM tables (A: emb at partitions 64:96; B: emb at partitions 0:32)
    so the per-step hidden-state writes land partition-aligned directly in
    the g tiles -> the 4 partition-shifting stream copies per step are
    replaced by 2 aligned 64-partition muls.
  * Cell state c in bf16 -> all DVE tensor_tensor ops run in 2x mode.
  * 4 SWDGE queues, gathers round-robin.
  * f-gate activation skipped at t=0 (c0 = i*g).
  * tanh(c) reads bf16 SBUF (scalar-engine accel).

Layouts:
  g_a tile [128, T*NT]: [h_fw 0:32 | h_bw 32:64 | emb 64:96 | embrev 96:128]
  g_b tile [128, T*NT]: [emb 0:32 | embrev 32:64 | h_fw 64:96 | h_bw 96:128]
  zt[gate] PSUM [128, NT]: [fwa 0:32 | bwa 32:64 | fwb 64:96 | bwb 96:128]
  st128 [128, ilv*NT]: same partition layout as zt; col = pair-local stream.
"""

import os
import numpy as np
import ml_dtypes

os.environ.setdefault("JAX_PLATFORMS", "cpu")

import concourse.bass as bass
import concourse.bacc as bacc
import concourse.mybir as mybir
import concourse.tile as tile
from concourse import bass_utils

BF16 = ml_dtypes.bfloat16

B, P, L = 2048, 64, 16
H = 32
VOC = 10000
VOCP = 10112                # 79 * 128
N_CORES = 8
B_LOC = B // N_CORES
NSEQ = B_LOC * P            # 16384 sequences per core
NT = 512
T = L


def _f32(x):
    return np.asarray(x, np.float32)


# ---------------------------------------------------------------------------
# device kernel
# ---------------------------------------------------------------------------

def build_kernel(nseq=NSEQ, mode="full", reps=1, ilv=2, nqueues=1,
                 c_dtype="bf16"):
    ntiles = nseq // NT
    npairs = ntiles // 2
    nsuper = npairs // ilv
    assert nsuper * ilv == npairs
    nbatch = nseq // P
    bat_sup = 2 * ilv * NT // P          # batches per super-iteration

    nc = bacc.Bacc("TRN2", target_bir_lowering=False, debug=False,
                   enable_asserts=False, num_devices=N_CORES,
                   num_swdge_queues=nqueues)
    qctr = [0]

    emb_d = nc.dram_tensor("emb", [128, ntiles * T * NT],
                           mybir.dt.bfloat16, kind="ExternalInput")
    wbd_d = nc.dram_tensor("wbd", [128, 512], mybir.dt.bfloat16,
                           kind="ExternalInput")
    bv_d = nc.dram_tensor("bv", [128, 4], mybir.dt.float32,
                          kind="ExternalInput")
    wd_d = nc.dram_tensor("wd", [128, 4096], mybir.dt.bfloat16,
                          kind="ExternalInput")
    bd_d = nc.dram_tensor("bd", [64, 1], mybir.dt.float32,
                          kind="ExternalInput")
    out_d = nc.dram_tensor("out", [64, nbatch], mybir.dt.float32,
                           kind="ExternalOutput")
    state_d = nc.dram_tensor("state", [128, nseq // 2], mybir.dt.bfloat16,
                             kind="ExternalOutput")

    FP32 = mybir.dt.float32
    BF = mybir.dt.bfloat16
    CDT = BF if c_dtype == "bf16" else FP32
    SIG = mybir.ActivationFunctionType.Sigmoid
    TANH = mybir.ActivationFunctionType.Tanh
    IDXW = NT * T // 16                  # idx cols per tile
    SUPW = 2 * ilv * IDXW                # idx cols per super-iteration

    with tile.TileContext(nc) as tc:
        with tc.tile_pool(name="const", bufs=1) as cpool:
            wbd = cpool.tile([128, 512], BF)
            nc.sync.dma_start(out=wbd[:, :], in_=wbd_d.ap())
            bv = cpool.tile([128, 4], FP32)
            nc.sync.dma_start(out=bv[:, :], in_=bv_d.ap())
            wd = cpool.tile([128, 4096], BF)
            nc.sync.dma_start(out=wd[:, :], in_=wd_d.ap())
            bd = cpool.tile([64, 1], FP32)
            nc.sync.dma_start(out=bd[:, :], in_=bd_d.ap())

            gbufs = 2 if ilv <= 2 else 1
            with tc.tile_pool(name="gbuf", bufs=gbufs) as gpool, \
                 tc.tile_pool(name="stage", bufs=2) as spool, \
                 tc.tile_pool(name="work", bufs=1) as wpool, \
                 tc.tile_pool(name="zps", bufs=(1 if ilv >= 4 else 2),
                              space="PSUM") as zpool:

                def gather_tile(j, tag, is_b):
                    g = gpool.tile([128, 1, T * NT], BF, tag=tag,
                                   name=f"g_{j}")
                    if mode == "compute":
                        return g
                    QC = T * NT // 4
                    for qq in range(4):
                        nc.sync.dma_start(
                            out=g[:, :, qq * QC:(qq + 1) * QC],
                            in_=emb_d.ap()[:, j * T * NT + qq * QC:
                                           j * T * NT + (qq + 1) * QC])
                    return g

                def emit_step2(gq, c2, st_sup, tau, kf, sup):
                    # two chains (k2 = 2*kf, 2*kf+1) fused at FD=2*NT
                    N2 = 2 * NT
                    zt = [zpool.tile([128, N2], FP32, tag=f"z{gi}",
                                     name=f"z{gi}_{kf}_{tau}_{sup}")
                          for gi in range(4)]
                    for cc in range(2):
                        for ti in range(2):
                            g_t = gq[2 * cc + ti]
                            rhs = g_t[0:128, 0, tau * NT:(tau + 1) * NT]
                            for gi in range(4):
                                if tau == 0 and gi == 1:
                                    continue
                                nc.tensor.matmul(
                                    zt[gi][64 * ti:64 * ti + 64,
                                           cc * NT:(cc + 1) * NT],
                                    wbd[:, 256 * ti + 64 * gi:
                                        256 * ti + 64 * gi + 64], rhs,
                                    start=True, stop=True,
                                    tile_position=(0, 64 * ti))

                    ui = wpool.tile([128, N2], BF, tag=f"ui{kf}", bufs=2)
                    nc.scalar.activation(ui[:, :], zt[0][:, :], SIG,
                                         bias=bv[:, 0:1])
                    if tau > 0:
                        uf = wpool.tile([128, N2], BF, tag=f"uf{kf}")
                        nc.scalar.activation(uf[:, :], zt[1][:, :], SIG,
                                             bias=bv[:, 1:2])
                    g_all = wpool.tile([128, N2], BF, tag=f"ug{kf}",
                                       bufs=2)
                    nc.scalar.activation(g_all[:, :], zt[2][:, :], TANH,
                                         bias=bv[:, 2:3])
                    uo = wpool.tile([128, N2], BF, tag=f"uo{kf}")
                    nc.scalar.activation(uo[:, :], zt[3][:, :], SIG,
                                         bias=bv[:, 3:4])

                    if tau == 0:
                        nc.vector.tensor_mul(c2[:, :], ui[:, :], g_all[:, :])
                    else:
                        t1 = wpool.tile([128, N2], CDT, tag=f"t1{kf}")
                        nc.vector.tensor_mul(t1[:, :], ui[:, :], g_all[:, :])
                        t2 = wpool.tile([128, N2], CDT, tag=f"t2{kf}")
                        nc.vector.tensor_mul(t2[:, :], uf[:, :], c2[:, :])
                        nc.vector.tensor_add(c2[:, :], t1[:, :], t2[:, :])

                    tc_t = wpool.tile([128, N2], BF, tag=f"tc{kf}")
                    nc.scalar.activation(tc_t[:, :], c2[:, :], TANH)

                    if tau == T - 1:
                        s0 = 2 * kf * NT
                        nc.vector.tensor_mul(st_sup[0:64, s0:s0 + N2],
                                             uo[0:64, :], tc_t[0:64, :])
                        nc.vector.tensor_mul(st_sup[64:128, s0:s0 + N2],
                                             uo[64:128, :], tc_t[64:128, :])
                    else:
                        cb = (tau + 1) * NT
                        for cc in range(2):
                            c0 = cc * NT
                            nc.vector.tensor_mul(
                                gq[2 * cc][0:64, 0, cb:cb + NT],
                                uo[0:64, c0:c0 + NT],
                                tc_t[0:64, c0:c0 + NT])
                            nc.vector.tensor_mul(
                                gq[2 * cc + 1][64:128, 0, cb:cb + NT],
                                uo[64:128, c0:c0 + NT],
                                tc_t[64:128, c0:c0 + NT])

                def emit_step(gs, c_all, st_sup, tau, pair, k2, sup):
                    zt = [zpool.tile([128, NT], FP32, tag=f"z{gi}",
                                     name=f"z{gi}_{pair}_{tau}")
                          for gi in range(4)]
                    for ti in range(2):
                        g_t = gs[ti]
                        rhs = g_t[0:128, 0, tau * NT:(tau + 1) * NT]
                        for gi in range(4):   # gate order i,f,g,o
                            if tau == 0 and gi == 1:
                                continue      # f unused at t=0
                            nc.tensor.matmul(
                                zt[gi][64 * ti:64 * ti + 64, :],
                                wbd[:, 256 * ti + 64 * gi:
                                    256 * ti + 64 * gi + 64], rhs,
                                start=True, stop=True,
                                tile_position=(0, 64 * ti))

                    ui = wpool.tile([128, NT], BF, tag=f"ui{k2}")
                    nc.scalar.activation(ui[:, :], zt[0][:, :], SIG,
                                         bias=bv[:, 0:1])
                    if tau > 0:
                        uf = wpool.tile([128, NT], BF, tag=f"uf{k2}")
                        nc.scalar.activation(uf[:, :], zt[1][:, :], SIG,
                                             bias=bv[:, 1:2])
                    g_all = wpool.tile([128, NT], BF, tag=f"ug{k2}")
                    nc.scalar.activation(g_all[:, :], zt[2][:, :], TANH,
                                         bias=bv[:, 2:3])
                    uo = wpool.tile([128, NT], BF, tag=f"uo{k2}")
                    nc.scalar.activation(uo[:, :], zt[3][:, :], SIG,
                                         bias=bv[:, 3:4])

                    if tau == 0:
                        nc.vector.tensor_mul(c_all[:, :], ui[:, :],
                                             g_all[:, :])
                    else:
                        t1 = wpool.tile([128, NT], CDT, tag=f"t1{k2}")
                        nc.vector.tensor_mul(t1[:, :], ui[:, :], g_all[:, :])
                        t2 = wpool.tile([128, NT], CDT, tag=f"t2{k2}")
                        nc.vector.tensor_mul(t2[:, :], uf[:, :], c_all[:, :])
                        nc.vector.tensor_add(c_all[:, :], t1[:, :], t2[:, :])

                    tc_t = wpool.tile([128, NT], BF, tag=f"tc{k2}")
                    nc.scalar.activation(tc_t[:, :], c_all[:, :], TANH)

                    # direct partition-aligned h writes
                    if tau == T - 1:
                        d0 = st_sup[0:64, k2 * NT:(k2 + 1) * NT]
                        d1 = st_sup[64:128, k2 * NT:(k2 + 1) * NT]
                    else:
                        cb = (tau + 1) * NT
                        d0 = gs[0][0:64, 0, cb:cb + NT]
                        d1 = gs[1][64:128, 0, cb:cb + NT]
                    nc.vector.tensor_mul(d0, uo[0:64, :], tc_t[0:64, :])
                    nc.vector.tensor_mul(d1, uo[64:128, :], tc_t[64:128, :])

                rep_ctx = tc.For_i(0, reps, 1) if reps > 1 else None
                if rep_ctx is not None:
                    rep_ctx.__enter__()
                for sup in range(nsuper):
                    if mode == "empty":
                        break
                    st_sup = spool.tile([128, ilv * NT], BF, tag="st",
                                        name=f"st_{sup}")
                    gs_k = []
                    for k2 in range(ilv):
                        pair = sup * ilv + k2
                        ga = gather_tile(2 * pair, f"g{2 * k2}",
                                         is_b=False)
                        gb = gather_tile(2 * pair + 1, f"g{2 * k2 + 1}",
                                         is_b=True)
                        gs_k.append([ga, gb])
                    if mode == "gather":
                        for k2 in range(ilv):
                            nc.vector.tensor_copy(
                                st_sup[0:32, k2 * NT:(k2 + 1) * NT],
                                gs_k[k2][0][64:96, 0, (T - 1) * NT:T * NT])
                        nc.sync.dma_start(
                            out=state_d.ap()[:, sup * ilv * NT:
                                             (sup + 1) * ilv * NT],
                            in_=st_sup[:, :])
                        continue
                    if ilv >= 4:
                        nfuse = ilv // 2
                        c_f = [wpool.tile([128, 2 * NT], CDT,
                                          tag=f"c{kf}", bufs=2,
                                          name=f"c_{sup}_{kf}")
                               for kf in range(nfuse)]
                        for tau in range(T):
                            for kf in range(nfuse):
                                gq = [gs_k[2 * kf][0], gs_k[2 * kf][1],
                                      gs_k[2 * kf + 1][0],
                                      gs_k[2 * kf + 1][1]]
                                emit_step2(gq, c_f[kf], st_sup, tau, kf,
                                           sup)
                    else:
                        c_k = [wpool.tile([128, NT], CDT, tag=f"c{k2}",
                                          bufs=2, name=f"c_{sup}_{k2}")
                               for k2 in range(ilv)]
                        for tau in range(T):
                            for k2 in range(ilv):
                                emit_step(gs_k[k2], c_k[k2], st_sup, tau,
                                          sup * ilv + k2, k2, sup)

                    nc.sync.dma_start(
                        out=state_d.ap()[:, sup * ilv * NT:
                                         (sup + 1) * ilv * NT],
                        in_=st_sup[:, :])

                    # dense head: a-tiles from partitions 0:64, b-tiles
                    # from 64:128. out_sb col = (k2, ab, b) = batch order.
                    st_r = st_sup[:, :].rearrange("p (k b q) -> p k b q",
                                                  k=ilv, q=P)
                    hpa = zpool.tile([64, 8 * ilv], FP32, tag="z0",
                                     name=f"hpa_{sup}")
                    hpb = zpool.tile([64, 8 * ilv], FP32, tag="z1",
                                     name=f"hpb_{sup}")
                    for p in range(P):
                        nc.tensor.matmul(hpa[0:64, :],
                                         wd[0:64, 64 * p:64 * p + 64],
                                         st_r[0:64, :, :, p:p + 1],
                                         start=(p == 0), stop=(p == P - 1))
                        nc.tensor.matmul(hpb[0:64, :],
                                         wd[64:128, 64 * p:64 * p + 64],
                                         st_r[64:128, :, :, p:p + 1],
                                         start=(p == 0), stop=(p == P - 1))
                    out_sb = spool.tile([64, bat_sup], FP32, tag="out",
                                        name=f"out_{sup}")
                    o_r = out_sb[:, :].rearrange("p (k a b) -> p k a b",
                                                 k=ilv, a=2)
                    nc.scalar.activation(o_r[:, :, 0, :],
                                         hpa[:, :].rearrange(
                                             "p (k b) -> p k b", k=ilv),
                                         TANH, bias=bd[:, :])
                    nc.scalar.activation(o_r[:, :, 1, :],
                                         hpb[:, :].rearrange(
                                             "p (k b) -> p k b", k=ilv),
                                         TANH, bias=bd[:, :])
                    nc.sync.dma_start(
                        out=out_d.ap()[:, sup * bat_sup:(sup + 1) * bat_sup],
                        in_=out_sb[:, :])

                if rep_ctx is not None:
                    rep_ctx.__exit__(None, None, None)

    nc.compile()
    return nc


# ---------------------------------------------------------------------------
# host-side packing
# ---------------------------------------------------------------------------

def pack_emb(table_bf, x_core, nseq=NSEQ):
    """Host-side embed + time-reverse + tile layout.

    Returns [128, ntiles*T*NT] bf16: per tile j, columns j*8192+(t*NT+n);
    a-tiles (j even): emb at partitions 64:96, reversed emb at 96:128;
    b-tiles (j odd):  emb at partitions 0:32,  reversed emb at 32:64;
    all other partitions zero (h rows; col 0 zeros = h0 init).
    """
    ntiles = nseq // NT
    e = table_bf[x_core]                       # [nseq, T, 32] bf16
    e = e.reshape(ntiles, NT, T, 32)
    eT = np.ascontiguousarray(e.transpose(0, 3, 2, 1))     # [j, d, t, n]
    erT = np.ascontiguousarray(e[:, :, ::-1, :].transpose(0, 3, 2, 1))
    blocks = np.zeros((ntiles, 128, T, NT), BF16)
    blocks[0::2, 64:96] = eT[0::2]
    blocks[0::2, 96:128] = erT[0::2]
    blocks[1::2, 0:32] = eT[1::2]
    blocks[1::2, 32:64] = erT[1::2]
    return np.ascontiguousarray(
        blocks.reshape(ntiles, 128, T * NT).transpose(1, 0, 2)
        .reshape(128, ntiles * T * NT))


def pack_weights(Wk, Wr, b):
    """Per-(tile-role, gate) weight blocks [128, 2*4*64] and bias [128, 4].

    wbd[:, 256*ti + 64*gi : +64] is the block for tile-role ti, gate gi.
    """
    Wk, Wr, b = _f32(Wk), _f32(Wr), _f32(b)
    wbd = np.zeros((128, 512), np.float32)
    for gi in range(4):
        wr = Wr[:, 32 * gi:32 * gi + 32]
        wk = Wk[:, 32 * gi:32 * gi + 32]
        # tile a: fmap [h_fw | h_bw | emb | embrev]
        blk = wbd[:, 64 * gi:64 * gi + 64]
        blk[0:32, 0:32] = wr
        blk[64:96, 0:32] = wk
        blk[32:64, 32:64] = wr
        blk[96:128, 32:64] = wk
        # tile b: fmap [emb | embrev | h_fw | h_bw]
        blk = wbd[:, 256 + 64 * gi:256 + 64 * gi + 64]
        blk[64:96, 0:32] = wr
        blk[0:32, 0:32] = wk
        blk[96:128, 32:64] = wr
        blk[32:64, 32:64] = wk
    bv = np.tile(b.reshape(4, 32), (1, 4)).reshape(4, 128).T
    return wbd.astype(BF16), np.ascontiguousarray(bv, np.float32)


def pack_wd(Wd):
    w = _f32(Wd).reshape(P, 64, 64).transpose(1, 0, 2).reshape(64, 4096)
    return np.concatenate([w, w], axis=0).astype(BF16)


# ---------------------------------------------------------------------------
# host reference bits for the zero-token fixup
# ---------------------------------------------------------------------------

def _np_lstm_last_h(emb, mask, Wk, Wr, b):
    n = emb.shape[0]
    h = np.zeros((n, H), np.float32)
    c = np.zeros((n, H), np.float32)
    for t in range(emb.shape[1]):
        z = emb[:, t, :] @ Wk + h @ Wr + b
        i = 1.0 / (1.0 + np.exp(-z[:, 0:32]))
        f = 1.0 / (1.0 + np.exp(-z[:, 32:64]))
        g = np.tanh(z[:, 64:96])
        o = 1.0 / (1.0 + np.exp(-z[:, 96:128]))
        c_new = f * c + i * g
        h_new = o * np.tanh(c_new)
        m = mask[:, t][:, None]
        h = np.where(m, h_new, h)
        c = np.where(m, c_new, c)
    return h


def _host_fixup(out, state_all, x_flat, embed_table, Wk, Wr, b, Wd, bd):
    mask = x_flat != 0
    bad_seq = np.nonzero(~mask.all(axis=1))[0]
    if bad_seq.size == 0:
        return out
    emb = _f32(embed_table)[x_flat[bad_seq]]
    h_fw = _np_lstm_last_h(emb, mask[bad_seq], _f32(Wk), _f32(Wr), _f32(b))
    h_bw = _np_lstm_last_h(emb[:, ::-1, :], mask[bad_seq][:, ::-1],
                           _f32(Wk), _f32(Wr), _f32(b))
    state_all = state_all.copy()
    state_all[bad_seq] = np.concatenate([h_fw, h_bw], axis=1)
    bad_rows = np.unique(bad_seq // P)
    st = state_all[bad_rows[:, None] * P + np.arange(P)[None, :]]
    st = st.reshape(bad_rows.size, P * 64)
    out[bad_rows] = np.tanh(st @ _f32(Wd) + _f32(bd))
    return out


# ---------------------------------------------------------------------------
# entry point
# ---------------------------------------------------------------------------

_NC_CACHE = {}


def _get_nc(mode="full", reps=1, ilv=4, nqueues=1):
    key = f"nc{mode}{reps}{ilv}{nqueues}"
    if key not in _NC_CACHE:
        _NC_CACHE[key] = build_kernel(mode=mode, reps=reps, ilv=ilv,
                                      nqueues=nqueues)
    return _NC_CACHE[key]


def unpack_state(state_raw, nseq=NSEQ):
    """state_raw [128, nseq//2] -> state_all [nseq, 64] float32."""
    ntiles = nseq // NT
    st = _f32(state_raw).reshape(128, ntiles // 2, NT)
    out = np.empty((nseq, 64), np.float32)
    for pair in range(ntiles // 2):
        a0 = (2 * pair) * NT
        out[a0:a0 + NT] = st[0:64, pair, :].T
        b0 = (2 * pair + 1) * NT
        out[b0:b0 + NT] = st[64:128, pair, :].T
    return out


def run_device(inputs, trace=False, reps=1, ilv=4, nqueues=1):
    x = np.asarray(inputs["x"])
    table_bf = _f32(inputs["embed_table"]).astype(BF16)
    wbd, bv = pack_weights(inputs["Wk"], inputs["Wr"], inputs["b"])
    wd = pack_wd(inputs["Wd"])
    bd = _f32(inputs["bd"]).reshape(64, 1)

    x_flat = x.reshape(B * P, L)
    in_maps = []
    for k in range(N_CORES):
        x_core = x_flat[k * NSEQ:(k + 1) * NSEQ]
        in_maps.append({
            "emb": pack_emb(table_bf, x_core),
            "wbd": wbd,
            "bv": bv,
            "wd": wd,
            "bd": bd,
        })

    nc = _get_nc(reps=reps, ilv=ilv, nqueues=nqueues)
    res = bass_utils.run_bass_kernel_spmd(
        nc, in_maps, core_ids=list(range(N_CORES)), trace=trace)

    out = np.empty((B, 64), np.float32)
    state_all = np.empty((B * P, 64), np.float32)
    for k in range(N_CORES):
        out[k * B_LOC:(k + 1) * B_LOC] = res.results[k]["out"].T
        state_all[k * NSEQ:(k + 1) * NSEQ] = \
            unpack_state(res.results[k]["state"])
    return out, state_all, res


def kernel(x, embed_table, Wk, Wr, b, Wd, bd):
    inputs = dict(x=x, embed_table=embed_table, Wk=Wk, Wr=Wr, b=b,
                  Wd=Wd, bd=bd)
    out, state_all, _ = run_device(inputs)
    out = _host_fixup(out, state_all, np.asarray(x).reshape(B * P, L),
                      embed_table, Wk, Wr, b, Wd, bd)
    return out
